# revision 4
# baseline (speedup 1.0000x reference)
"""Fused dual-stream sliding-window attention for Trainium2 (Bass/Tile).

The reference computes two banded softmax streams (s: 0<=i-j<W, c: W<=i-j<2W)
and merges them via LSE. Over disjoint key sets that merge is exactly one
softmax over the union band 0 <= i-j < 2W (W=256), so we compute a single
fused banded attention.

Layout strategy (per (batch, head) pair, sharded 4 pairs/core x 8 cores):
  - host pre-transposes Q, K to [D=128, S] so the kernel never transposes
  - per query block b (256 rows), context = key blocks [b-2, b-1, b]
    = 6 chunks of 128 keys, computed in S^T orientation [ck, q]:
        S^T_chunk = matmul(lhsT=K^T[:, chunk], rhs=Q^T[:, block])   # [128, 256]
        p^T = exp(S^T * D^-0.5)        (ACT, fused scale)
        p^T *= triangle mask           (DVE, chunks 0/1/4/5 only)
        out^T accum: matmul(lhsT=p^T[:, half], rhs=V_aug[chunk])   # [128, 256]
    V_aug has a ones column at 128 so psum col 128 accumulates the softmax
    denominator; cols 129..255 are zero padding (keeps the moving free dim at
    256 where float32r matmuls run at full rate).
  - normalize with DVE reciprocal + tensor_scalar, DMA out.

All matmuls run in float32r (FP22 mantissa truncation, full PE rate at
free-dim 256) with fp32 accumulation.
"""

import numpy as np

import concourse.bass as bass
from concourse import bacc
import concourse.mybir as mybir
import concourse.tile as tile
from concourse.bass_utils import run_bass_kernel_spmd

B, S, H, D = 2, 2048, 16, 128
WIN = 256
N_CORES = 8
PAIRS = (B * H) // N_CORES          # 4 (batch, head) pairs per core
NB = S // WIN                       # 8 query blocks per sequence
NG = S // 128                       # 16 key subtiles of 128 per sequence
SCALE = float(D) ** -0.5
F32 = mybir.dt.float32
F32R = mybir.dt.float32r

# chunk -> mask slot in the [128, 4, 256] mask tensor (None = unmasked)
MASK_SLOT = {0: 0, 1: 1, 4: 2, 5: 3}
# (chunk, half) subtiles that are entirely masked out -> skip their PV matmul
EMPTY_SUBTILES = {(0, 1), (5, 0)}


def build_masks() -> np.ndarray:
    """0/1 triangle masks in the S^T layout: partition p = key-in-chunk,
    free f = query-in-block.  Valid band: f - p in [128*c - 512, 128*c - 1]."""
    p = np.arange(128)[:, None]
    f = np.arange(256)[None, :]
    m = np.zeros((128, 4, 256), np.float32)
    m[:, 0, :] = f < p            # chunk 0
    m[:, 1, :] = f < p + 128      # chunk 1
    m[:, 2, :] = f >= p           # chunk 4
    m[:, 3, :] = f >= p + 128     # chunk 5
    return m


def chunks_for_block(b: int) -> list[int]:
    # chunk c of query block b reads key subtile g = 2b - 4 + c; g must be >= 0
    return list(range(max(0, 4 - 2 * b), 6))


def build_program() -> bacc.Bacc:
    nc = bacc.Bacc("TRN2", target_bir_lowering=False, debug=False)

    qt = nc.dram_tensor("qt", [PAIRS, 128, S], F32R, kind="ExternalInput").ap()
    kt = nc.dram_tensor("kt", [PAIRS, 128, S], F32R, kind="ExternalInput").ap()
    vv = nc.dram_tensor("v", [PAIRS, S, 128], F32R, kind="ExternalInput").ap()
    mk = nc.dram_tensor("masks", [128, 4, 256], F32, kind="ExternalInput").ap()
    out = nc.dram_tensor("out", [PAIRS, S, 128], F32, kind="ExternalOutput").ap()

    with tile.TileContext(nc) as tc:
        with (
            tc.tile_pool(name="const", bufs=1) as const_pool,
            tc.tile_pool(name="qk", bufs=2) as q_pool,
            tc.tile_pool(name="kk", bufs=2) as k_pool,
            tc.tile_pool(name="vp", bufs=2) as v_pool,
            tc.tile_pool(name="pt", bufs=8) as pt_pool,
            tc.tile_pool(name="st", bufs=3, space="PSUM") as st_pool,
            tc.tile_pool(name="pv", bufs=2, space="PSUM") as pv_pool,
            tc.tile_pool(name="outp", bufs=4) as out_pool,
            tc.tile_pool(name="rcp", bufs=4) as rcp_pool,
        ):
            mask_sb = const_pool.tile([128, 4, 256], F32)
            nc.sync.dma_start(mask_sb[:], mk[:])

            for pair in range(PAIRS):
                qt_sb = q_pool.tile([128, S], F32R)
                nc.sync.dma_start(qt_sb[:], qt[pair])
                kt_sb = k_pool.tile([128, S], F32R)
                nc.sync.dma_start(kt_sb[:], kt[pair])

                # V in natural layout + ones column at 128 + zero pad to 256
                v_sb = v_pool.tile([128, NG, 256], F32R)
                nc.gpsimd.memset(v_sb[:, :, 128:129].bitcast(F32), 1.0)
                nc.gpsimd.memset(v_sb[:, :, 129:256].bitcast(F32), 0.0)
                nc.sync.dma_start(
                    v_sb[:, :, 0:128],
                    vv[pair].rearrange("(g p) d -> p g d", p=128),
                )

                for b in range(NB):
                    cs = chunks_for_block(b)
                    q_mv = qt_sb[:, b * 256:(b + 1) * 256]

                    pts = {}
                    for c in cs:
                        g = 2 * b - 4 + c
                        st = st_pool.tile([128, 256], F32)
                        nc.tensor.matmul(
                            st[:],
                            lhsT=kt_sb[:, g * 128:(g + 1) * 128],
                            rhs=q_mv,
                            start=True,
                            stop=True,
                        )
                        pt = pt_pool.tile([128, 256], F32R)
                        nc.scalar.activation(
                            pt[:], st[:], mybir.ActivationFunctionType.Exp,
                            scale=SCALE,
                        )
                        slot = MASK_SLOT.get(c)
                        if slot is not None:
                            nc.vector.tensor_mul(pt[:], pt[:], mask_sb[:, slot, :])
                        pts[c] = pt

                    pv = pv_pool.tile([128, 2, 256], F32)
                    for h in (0, 1):
                        mms = [c for c in cs if (c, h) not in EMPTY_SUBTILES]
                        for i, c in enumerate(mms):
                            g = 2 * b - 4 + c
                            nc.tensor.matmul(
                                pv[:, h, :],
                                lhsT=pts[c][:, h * 128:(h + 1) * 128],
                                rhs=v_sb[:, g, :],
                                start=(i == 0),
                                stop=(i == len(mms) - 1),
                            )

                    recip = rcp_pool.tile([128, 2], F32)
                    nc.vector.reciprocal(recip[:], pv[:, :, 128])
                    ot = out_pool.tile([128, 2, 128], F32)
                    for h in (0, 1):
                        nc.vector.tensor_scalar_mul(
                            ot[:, h, :], pv[:, h, 0:128], recip[:, h:h + 1]
                        )
                    nc.sync.dma_start(
                        out[pair, b * 256:(b + 1) * 256, :].rearrange(
                            "(h p) d -> p h d", h=2
                        ),
                        ot[:],
                    )
    nc.compile()
    return nc


_CACHE: dict = {}


def _get_program() -> bacc.Bacc:
    if "nc" not in _CACHE:
        _CACHE["nc"] = build_program()
    return _CACHE["nc"]


def make_in_maps(query, key, value):
    """Shard + pre-transpose full [B,S,H,D] inputs into per-core input maps."""
    qt_all = np.ascontiguousarray(query.transpose(0, 2, 3, 1))  # [B,H,D,S]
    kt_all = np.ascontiguousarray(key.transpose(0, 2, 3, 1))
    v_all = np.ascontiguousarray(value.transpose(0, 2, 1, 3))   # [B,H,S,D]
    masks = build_masks()
    in_maps = []
    for c in range(N_CORES):
        idx = [divmod(c * PAIRS + i, H) for i in range(PAIRS)]
        in_maps.append({
            "qt": np.stack([qt_all[b, h] for b, h in idx]),
            "kt": np.stack([kt_all[b, h] for b, h in idx]),
            "v": np.stack([v_all[b, h] for b, h in idx]),
            "masks": masks,
        })
    return in_maps


def gather_output(results) -> np.ndarray:
    out = np.empty((B, S, H, D), np.float32)
    for c in range(N_CORES):
        o = results[c]["out"]
        for i in range(PAIRS):
            b, h = divmod(c * PAIRS + i, H)
            out[b, :, h, :] = o[i]
    return out


def run(query, key, value, trace: bool = False):
    nc = _get_program()
    in_maps = make_in_maps(query, key, value)
    res = run_bass_kernel_spmd(nc, in_maps, core_ids=list(range(N_CORES)),
                               trace=trace)
    return gather_output(res.results), res


def kernel(query, key, value):
    out, _ = run(query, key, value)
    return out
